# revision 3
# baseline (speedup 1.0000x reference)
"""Trainium2 Bass kernel for ColumnMixedPrecisionLinear (v3).

Computes out[b,s,o] = bias[o] + sum_i x_i[b,s,:] @ (wq_i * s_i[:,None]).T
where x is [4, 2048, 4096] fp32, wq_i are [4096, 1024] int8 slices of the
weight along the input dim, s_i are per-output-channel scales.

Strategy: data-parallel over tokens across 8 NeuronCores. Each core gets
1024 tokens of x (flattened [8192, 4096]) and the full weights, and computes
its [1024, 4096] output shard (stored transposed, host un-transposes).

v3 vs v2: all transposes moved to the HOST (pure layout prep, like v2's
scale/bias rearrange). No DRAM scratch round trips, no xbar/PE transposes:
  - x uploaded pre-transposed per core: xt [4096d, 1024t] bf16 -> straight
    HWDGE load into SBUF d-major quarter tiles.
  - weights uploaded as one concatenated pre-transposed tensor
    wqt [4096d, 4096o] int8 -> per 512-o-chunk SWDGE cast DMA int8->bf16,
    dequantized in place on DVE by per-slice scale rows (bf16, broadcast
    along free dim).
  - matmul computes the output TRANSPOSED: psum[128o, 512t] =
    wt_blk[128d,128o].T @ xT[128d, 512t], accumulated over 32 d-blocks.
    Each stationary weight block serves 2 moving matmuls (token halves).
  - bias is added during the PSUM->SBUF drain on ACT (per-partition fp32
    bias operand of activation(Identity)) -- no ones-matmul.
Per-core DRAM traffic ~46 MB (v2: ~134 MB); PE does only the 2048 main
matmuls (v2: 2112).

int8 weights are exact in bf16; x rounds once to bf16, scales round to
bf16, w*s product rounds to bf16; output rel err ~3e-3.
"""

import numpy as np
import ml_dtypes

import concourse.bass as bass
import concourse.mybir as mybir
import concourse.tile as tile
from concourse import bacc
from concourse.bass_utils import run_bass_kernel_spmd

P = 128
N_CORES = 8
B, S = 4, 2048
D_IN_SLICE = 1024
N_SLICES = 4
D = D_IN_SLICE * N_SLICES      # 4096 contraction dim
O = 4096                       # out features
T = (B * S) // N_CORES         # 1024 tokens per core

D_BLKS = D // P                # 32
D_BLKS_SLICE = D_IN_SLICE // P # 8
O_CHUNK = 512
O_CHUNKS = O // O_CHUNK        # 8
O_TILES_PER_CHUNK = O_CHUNK // P  # 4
T_HALF = T // 2                # 512 moving tokens per matmul
XQ = 4                         # x loaded in 4 quarter tiles (8 d-blocks each)

BF16 = mybir.dt.bfloat16
FP32 = mybir.dt.float32
INT8 = mybir.dt.int8


def build_nc():
    nc = bacc.Bacc(None, target_bir_lowering=False)

    # host-pretransposed inputs
    xt_in = nc.dram_tensor("xt", [D, T], BF16, kind="ExternalInput")
    wqt_in = nc.dram_tensor("wqt", [D, O], INT8, kind="ExternalInput")
    # scb[i][p, o] = bf16(s_i[o]) replicated across partitions
    scb_in = [
        nc.dram_tensor(f"scb{i}", [P, O], BF16, kind="ExternalInput")
        for i in range(N_SLICES)
    ]
    # biasc[p, G] = bias[G*128 + p] for global o-tile G
    biasc_in = nc.dram_tensor("biasc", [P, O // P], FP32, kind="ExternalInput")
    outT = nc.dram_tensor("outT", [O, T], FP32, kind="ExternalOutput")

    with tile.TileContext(nc) as tc:
        with (
            tc.tile_pool(name="const", bufs=1) as const,
            tc.tile_pool(name="xres", bufs=1) as xres,
            tc.tile_pool(name="wt", bufs=2) as wt_pool,
            tc.tile_pool(name="ostage", bufs=2) as ostage,
            tc.tile_pool(name="psm", bufs=4, space="PSUM") as psm,
        ):
            biasc = const.tile([P, O // P], FP32)
            nc.gpsimd.dma_start(biasc[:], biasc_in[:])
            scbs = []
            for i in range(N_SLICES):
                sct = const.tile([P, O], BF16, tag=f"scb{i}")
                nc.gpsimd.dma_start(sct[:], scb_in[i][:])
                scbs.append(sct)

            # x: [4096, 1024] bf16 -> 4 quarter tiles [128, 8, 1024],
            # d = (q*8 + db)*128 + p
            xTq = []
            for q in range(XQ):
                xq = xres.tile([P, D_BLKS // XQ, T], BF16, tag=f"xTq{q}",
                               name=f"xTq{q}")
                nc.sync.dma_start(
                    xq[:],
                    xt_in[q * (D // XQ):(q + 1) * (D // XQ), :]
                    .rearrange("(db p) t -> p db t", p=P),
                )
                xTq.append(xq)

            for c in range(O_CHUNKS):
                # weight chunk: [4096d, 512o] int8 -> [128, 32db, 512] bf16
                wt = wt_pool.tile([P, D_BLKS, O_CHUNK], BF16, tag="wt")
                nc.gpsimd.dma_start(
                    wt[:],
                    wqt_in[:, c * O_CHUNK:(c + 1) * O_CHUNK]
                    .rearrange("(db p) o -> p db o", p=P),
                )
                # dequant in place: slice i covers d-blocks i*8..i*8+7
                for i in range(N_SLICES):
                    nc.vector.tensor_tensor(
                        wt[:, i * D_BLKS_SLICE:(i + 1) * D_BLKS_SLICE, :],
                        wt[:, i * D_BLKS_SLICE:(i + 1) * D_BLKS_SLICE, :],
                        scbs[i][:, None, c * O_CHUNK:(c + 1) * O_CHUNK]
                        .to_broadcast((P, D_BLKS_SLICE, O_CHUNK)),
                        mybir.AluOpType.mult,
                    )

                ob = ostage.tile([P, O_TILES_PER_CHUNK, T], FP32, tag="ob")
                for g in range(O_TILES_PER_CHUNK):
                    ps0 = psm.tile([P, T_HALF], FP32, tag="ps0", name="ps0")
                    ps1 = psm.tile([P, T_HALF], FP32, tag="ps1", name="ps1")
                    ps = [ps0, ps1]
                    for db in range(D_BLKS):
                        lhsT = wt[:, db, g * P:(g + 1) * P]
                        for h in range(2):
                            nc.tensor.matmul(
                                ps[h][:],
                                lhsT,
                                xTq[db // D_BLKS_SLICE][
                                    :, db % D_BLKS_SLICE,
                                    h * T_HALF:(h + 1) * T_HALF],
                                start=(db == 0),
                                stop=(db == D_BLKS - 1),
                            )
                    G = c * O_TILES_PER_CHUNK + g
                    for h in range(2):
                        # drain + fp32 bias add on ACT
                        nc.scalar.activation(
                            ob[:, g, h * T_HALF:(h + 1) * T_HALF],
                            ps[h][:],
                            mybir.ActivationFunctionType.Identity,
                            bias=biasc[:, G:G + 1],
                            scale=1.0,
                        )
                # store chunk: outT rows o = (c*4 + g)*128 + p
                nc.sync.dma_start(
                    outT[c * O_CHUNK:(c + 1) * O_CHUNK, :]
                    .rearrange("(g p) t -> p g t", p=P),
                    ob[:],
                )
    nc.compile()
    return nc


_NC_CACHE = None


def _get_nc():
    global _NC_CACHE
    if _NC_CACHE is None:
        _NC_CACHE = build_nc()
    return _NC_CACHE


def _prep_inputs(x, wqs, ss, bias):
    xb = np.asarray(x, dtype=np.float32).reshape(B * S, D).astype(
        ml_dtypes.bfloat16)
    wqt = np.ascontiguousarray(
        np.concatenate(
            [np.asarray(w).astype(np.int8).T for w in wqs], axis=0))
    scbs = [
        np.ascontiguousarray(
            np.broadcast_to(
                np.asarray(s, dtype=np.float32).astype(ml_dtypes.bfloat16),
                (P, O)))
        for s in ss
    ]
    biasc = np.ascontiguousarray(
        np.asarray(bias, dtype=np.float32).reshape(O // P, P).T)
    in_maps = []
    for c in range(N_CORES):
        m = {
            "xt": np.ascontiguousarray(xb[c * T:(c + 1) * T, :].T),
            "wqt": wqt,
            "biasc": biasc,
        }
        for i in range(N_SLICES):
            m[f"scb{i}"] = scbs[i]
        in_maps.append(m)
    return in_maps


def run_on_hw(x, wqs, ss, bias, **spmd_kwargs):
    """Run and return (out_full [B,S,O] fp32, BassKernelResults)."""
    nc = _get_nc()
    in_maps = _prep_inputs(x, wqs, ss, bias)
    res = run_bass_kernel_spmd(nc, in_maps, core_ids=list(range(N_CORES)),
                               **spmd_kwargs)
    # each core returns outT [O, T]; tokens concatenate along axis 1
    out = np.concatenate([r["outT"] for r in res.results], axis=1)
    return np.ascontiguousarray(out.T.reshape(B, S, O).astype(np.float32)), res


def kernel(x, wq0, s0, wq1, s1, wq2, s2, wq3, s3, bias):
    out, _ = run_on_hw(x, [wq0, wq1, wq2, wq3], [s0, s1, s2, s3], bias)
    return out
